# revision 3
# baseline (speedup 1.0000x reference)
"""MeshGNN Trainium2 kernel.

Mathematical reduction: the reference broadcasts the text projection to all 12
mesh vertices, and the row-normalized kNN adjacency has identical row sums
(every vertex has exactly K_NN=6 neighbors), so node features remain identical
across vertices through every GNN layer.  The whole network therefore
collapses to a per-row MLP:

    h   = relu(x @ W0c + b0c)          W0c = W_text @ (s*W_gnn[0])  (384,256)
    h   = relu(h @ (s*W_gnn[l]) + b_gnn[l])   l = 1..3
    o36 = h @ W4c + b4c                W4c = tile(W_out, 12) (256,36)
    out = o36.reshape(B, 12, 3)        b4c = tile(b_out,12) + template.flat

where s = 6/(6+1e-6) is the common adjacency row sum.

Device strategy (8 cores, pure data parallel over the batch):
  - host pre-transposes each core's x shard to (384, 4096) so features sit on
    SBUF partitions; all matmuls then run in feature-on-partition layout with
    weights as the stationary operand and activations as the moving operand.
  - float32r matmuls (1 cycle/row at N=512) -> no casts, fp32 accuracy-ish.
  - relu+bias fused on ScalarE (activation) / VectorE (tensor_scalar),
    alternating layers to balance the two engines.
  - output computed as (36, 4096) on device; host transposes back.
"""

import os

import numpy as np

# ---------------------------------------------------------------- constants
B = 32768
CORES = 8
ROWS = B // CORES            # 4096 rows per core
TD = 384                     # text dim
H = 256                      # hidden
OUT = 36                     # 12 verts * 3 coords
NBLK = 8                     # row blocks per core
N = ROWS // NBLK             # 512 rows per block
KT0 = TD // 128              # 3 k-tiles for layer 0
KTH = H // 128               # 2 k-tiles for hidden layers
MT = H // 128                # 2 m-tiles for hidden outputs

_BUILT = {}                  # cache: compiled Bass module across calls


def _build_bass():
    """Build + compile the per-core Bass program (same NEFF on all cores)."""
    import concourse.bass as bass
    import concourse.mybir as mybir
    import concourse.tile as tile
    from concourse import bacc

    f32 = mybir.dt.float32
    f32r = mybir.dt.float32r
    RELU = mybir.ActivationFunctionType.Relu
    IDENT = mybir.ActivationFunctionType.Identity
    ADD = mybir.AluOpType.add
    MAX = mybir.AluOpType.max

    nc = bacc.Bacc(
        "TRN2",
        target_bir_lowering=False,
        debug=False,
        enable_asserts=False,
        num_devices=CORES,
    )

    xt_d = nc.dram_tensor("xt", (TD, ROWS), f32r, kind="ExternalInput")
    w0_d = nc.dram_tensor("w0", (TD, H), f32r, kind="ExternalInput")
    wl_d = [
        nc.dram_tensor(f"w{l}", (H, H), f32r, kind="ExternalInput") for l in (1, 2, 3)
    ]
    w4_d = nc.dram_tensor("w4", (H, OUT), f32r, kind="ExternalInput")
    bl_d = [
        nc.dram_tensor(f"b{l}", (128, MT), f32, kind="ExternalInput")
        for l in (0, 1, 2, 3)
    ]
    b4_d = nc.dram_tensor("b4", (OUT, 1), f32, kind="ExternalInput")
    out_d = nc.dram_tensor("out", (OUT, ROWS), f32, kind="ExternalOutput")

    # x viewed as (partition, ktile, row): row-major (TD, ROWS) split over 128
    xt_v = xt_d.ap().rearrange("(k p) n -> p k n", p=128)

    with tile.TileContext(nc) as tc:
        with (
            tc.tile_pool(name="wp", bufs=1) as wp,
            tc.tile_pool(name="xp", bufs=3) as xp,
            tc.tile_pool(name="hp", bufs=2) as hp,
            tc.tile_pool(name="op", bufs=3) as op,
            tc.tile_pool(name="pp", bufs=4, space="PSUM") as pp,
            tc.tile_pool(name="pp4", bufs=2, space="PSUM") as pp4,
        ):
            # ---- weights / biases, loaded once
            w0_t = {}
            for k in range(KT0):
                for m in range(MT):
                    t = wp.tile([128, 128], f32r, tag=f"w0_{k}_{m}")
                    nc.sync.dma_start(
                        t[:], w0_d.ap()[k * 128:(k + 1) * 128, m * 128:(m + 1) * 128]
                    )
                    w0_t[k, m] = t
            wl_t = {}
            for li, l in enumerate((1, 2, 3)):
                for k in range(KTH):
                    for m in range(MT):
                        t = wp.tile([128, 128], f32r, tag=f"w{l}_{k}_{m}")
                        nc.sync.dma_start(
                            t[:],
                            wl_d[li].ap()[
                                k * 128:(k + 1) * 128, m * 128:(m + 1) * 128
                            ],
                        )
                        wl_t[l, k, m] = t
            w4_t = {}
            for k in range(KTH):
                t = wp.tile([128, OUT], f32r, tag=f"w4_{k}")
                nc.sync.dma_start(t[:], w4_d.ap()[k * 128:(k + 1) * 128, :])
                w4_t[k] = t
            bl_t = {}
            for l in range(4):
                t = wp.tile([128, MT], f32, tag=f"b{l}")
                nc.sync.dma_start(t[:], bl_d[l].ap()[:])
                bl_t[l] = t
            b4_t = wp.tile([OUT, 1], f32, tag="b4")
            nc.sync.dma_start(b4_t[:], b4_d.ap()[:])

            # ---- main loop over 512-row blocks
            for blk in range(NBLK):
                xt = xp.tile([128, KT0, N], f32r, tag="x")
                nc.sync.dma_start(xt[:], xt_v[:, :, blk * N:(blk + 1) * N])

                h_prev = None
                for l in range(4):
                    w_tiles = (
                        {(k, m): w0_t[k, m] for k in range(KT0) for m in range(MT)}
                        if l == 0
                        else {(k, m): wl_t[l, k, m] for k in range(KTH) for m in range(MT)}
                    )
                    nk = KT0 if l == 0 else KTH
                    h_cur = hp.tile([128, MT, N], f32r, tag=f"h{l}")
                    for m in range(MT):
                        ps = pp.tile([128, N], f32, tag="ps")
                        for k in range(nk):
                            rhs = xt[:, k, :] if l == 0 else h_prev[:, k, :]
                            nc.tensor.matmul(
                                ps[:],
                                w_tiles[k, m][:],
                                rhs,
                                start=(k == 0),
                                stop=(k == nk - 1),
                            )
                        # relu(x + bias): alternate engines to balance load
                        if l % 2 == 0:
                            nc.scalar.activation(
                                h_cur[:, m, :], ps[:], RELU,
                                bias=bl_t[l][:, m:m + 1],
                            )
                        else:
                            nc.vector.tensor_scalar(
                                h_cur[:, m, :], ps[:],
                                bl_t[l][:, m:m + 1], 0.0, ADD, MAX,
                            )
                    h_prev = h_cur

                ps4 = pp4.tile([OUT, N], f32, tag="ps4")
                for k in range(KTH):
                    nc.tensor.matmul(
                        ps4[:],
                        w4_t[k][:],
                        h_prev[:, k, :],
                        start=(k == 0),
                        stop=(k == KTH - 1),
                    )
                ob = op.tile([OUT, N], f32, tag="ob")
                nc.scalar.activation(ob[:], ps4[:], IDENT, bias=b4_t[:])
                nc.sync.dma_start(out_d.ap()[:, blk * N:(blk + 1) * N], ob[:])

    nc.compile()
    return nc


def _fold_weights(W_text, b_text, W_gnn, b_gnn, W_out, b_out, adjacency, template):
    s_rows = adjacency.astype(np.float64).sum(axis=1)
    if np.ptp(s_rows) > 1e-5:
        raise ValueError("adjacency row sums are not uniform; collapse invalid")
    s = float(s_rows.mean())

    W0c = (W_text.astype(np.float64) @ (s * W_gnn[0].astype(np.float64)))
    b0c = s * (b_text.astype(np.float64) @ W_gnn[0].astype(np.float64)) + b_gnn[0]
    Wl = [s * W_gnn[l].astype(np.float64) for l in (1, 2, 3)]
    bl = [b_gnn[l] for l in (1, 2, 3)]
    W4c = np.tile(W_out, (1, 12))
    b4c = np.tile(b_out, 12) + template.reshape(36)

    def f32c(a):
        return np.ascontiguousarray(a, dtype=np.float32)

    biases = [f32c(np.asarray(b).reshape(MT, 128).T) for b in [b0c, *bl]]
    return (
        f32c(W0c), [f32c(w) for w in Wl], f32c(W4c),
        biases, f32c(np.asarray(b4c).reshape(OUT, 1)),
    )


def kernel(**inputs):
    from concourse.bass_utils import run_bass_kernel_spmd

    x = np.ascontiguousarray(np.asarray(inputs["text_emb"], dtype=np.float32))
    W0c, Wl, W4c, biases, b4c = _fold_weights(
        np.asarray(inputs["W_text"]), np.asarray(inputs["b_text"]),
        np.asarray(inputs["W_gnn"]), np.asarray(inputs["b_gnn"]),
        np.asarray(inputs["W_out"]), np.asarray(inputs["b_out"]),
        np.asarray(inputs["adjacency"]), np.asarray(inputs["template"]),
    )

    if "nc" not in _BUILT:
        _BUILT["nc"] = _build_bass()
    nc = _BUILT["nc"]

    in_maps = []
    for c in range(CORES):
        shard = np.ascontiguousarray(x[c * ROWS:(c + 1) * ROWS].T)
        m = {"xt": shard, "w0": W0c, "w4": W4c, "b4": b4c}
        for i, l in enumerate((1, 2, 3)):
            m[f"w{l}"] = Wl[i]
        for l in range(4):
            m[f"b{l}"] = biases[l]
        in_maps.append(m)

    res = run_bass_kernel_spmd(
        nc, in_maps, core_ids=list(range(CORES)),
        trace=bool(os.environ.get("MESHGNN_TRACE")),
    )
    _BUILT["last_results"] = res
    _BUILT["last_in_maps"] = in_maps

    full = np.empty((B, OUT), dtype=np.float32)
    for c in range(CORES):
        full[c * ROWS:(c + 1) * ROWS] = res.results[c]["out"].T
    return full.reshape(B, 12, 3)


# revision 4
# speedup vs baseline: 4.9892x; 4.9892x over previous
"""MeshGNN Trainium2 kernel.

Mathematical reduction: the reference broadcasts the text projection to all 12
mesh vertices, and the row-normalized kNN adjacency has identical row sums
(every vertex has exactly K_NN=6 neighbors), so node features remain identical
across vertices through every GNN layer.  The whole network therefore
collapses to a per-row MLP:

    h   = relu(x @ W0c + b0c)          W0c = W_text @ (s*W_gnn[0])  (384,256)
    h   = relu(h @ (s*W_gnn[l]) + b_gnn[l])   l = 1..3
    o36 = h @ W4c + b4c                W4c = tile(W_out, 12) (256,36)
    out = o36.reshape(B, 12, 3)        b4c = tile(b_out,12) + template.flat

where s = 6/(6+1e-6) is the common adjacency row sum.

Device strategy (8 cores, pure data parallel over the batch):
  - host pre-transposes each core's x shard to (384, 4096) so features sit on
    SBUF partitions; all matmuls then run in feature-on-partition layout with
    weights as the stationary operand and activations as the moving operand.
  - float32r (default) or bf16 matmuls, 1 PE cycle/row at N=512.
  - relu+bias fused into one op per layer (both m-tiles at once), alternating
    ScalarE / VectorE between layers to balance the two engines.
  - output computed as (36, 4096) on device; host transposes back.
"""

import os

import numpy as np

# ---------------------------------------------------------------- constants
B = 32768
CORES = 8
ROWS = B // CORES            # 4096 rows per core
TD = 384                     # text dim
H = 256                      # hidden
OUT = 36                     # 12 verts * 3 coords
NBLK = 8                     # row blocks per core
N = ROWS // NBLK             # 512 rows per block
KT0 = TD // 128              # 3 k-tiles for layer 0
KTH = H // 128               # 2 k-tiles for hidden layers
MT = H // 128                # 2 m-tiles for hidden outputs

MM_DTYPE = os.environ.get("MESHGNN_DTYPE", "f32r")   # "f32r" | "bf16"

_BUILT = {}                  # cache: compiled Bass modules keyed by config


def _np_mm_dtype():
    if MM_DTYPE == "bf16":
        import ml_dtypes
        return ml_dtypes.bfloat16
    return np.float32


def _build_bass(repeat=1):
    """Build + compile the per-core Bass program (same NEFF on all cores).

    repeat > 1 re-runs the whole pipeline that many times inside one NEFF
    (identical outputs each pass) -- used for dispatch-free HW timing.
    """
    import concourse.mybir as mybir
    import concourse.tile as tile
    from concourse import bacc

    f32 = mybir.dt.float32
    mmdt = mybir.dt.float32r if MM_DTYPE == "f32r" else mybir.dt.bfloat16
    RELU = mybir.ActivationFunctionType.Relu
    IDENT = mybir.ActivationFunctionType.Identity
    ADD = mybir.AluOpType.add
    MAX = mybir.AluOpType.max

    nc = bacc.Bacc(
        "TRN2",
        target_bir_lowering=False,
        debug=False,
        enable_asserts=False,
        num_devices=CORES,
    )

    xt_d = nc.dram_tensor("xt", (TD, ROWS), mmdt, kind="ExternalInput")
    w0_d = nc.dram_tensor("w0", (TD, H), mmdt, kind="ExternalInput")
    wl_d = [
        nc.dram_tensor(f"w{l}", (H, H), mmdt, kind="ExternalInput")
        for l in (1, 2, 3)
    ]
    w4_d = nc.dram_tensor("w4", (H, OUT), mmdt, kind="ExternalInput")
    bl_d = [
        nc.dram_tensor(f"b{l}", (128, MT), f32, kind="ExternalInput")
        for l in (0, 1, 2, 3)
    ]
    b4_d = nc.dram_tensor("b4", (OUT, 1), f32, kind="ExternalInput")
    out_d = nc.dram_tensor("out", (OUT, ROWS), f32, kind="ExternalOutput")

    # x viewed as (partition, ktile, row): row-major (TD, ROWS) split over 128
    xt_v = xt_d.ap().rearrange("(k p) n -> p k n", p=128)

    with tile.TileContext(nc) as tc:
        with (
            tc.tile_pool(name="wp", bufs=1) as wp,
            tc.tile_pool(name="xp", bufs=3) as xp,
            tc.tile_pool(name="hp", bufs=2) as hp,
            tc.tile_pool(name="op", bufs=3) as op,
            tc.tile_pool(name="pp", bufs=3, space="PSUM") as pp,
            tc.tile_pool(name="pp4", bufs=2, space="PSUM") as pp4,
        ):
            # ---- weights / biases, loaded once
            w0_t = {}
            for k in range(KT0):
                for m in range(MT):
                    t = wp.tile([128, 128], mmdt, tag=f"w0_{k}_{m}")
                    nc.sync.dma_start(
                        t[:], w0_d.ap()[k * 128:(k + 1) * 128, m * 128:(m + 1) * 128]
                    )
                    w0_t[k, m] = t
            wl_t = {}
            for li, l in enumerate((1, 2, 3)):
                for k in range(KTH):
                    for m in range(MT):
                        t = wp.tile([128, 128], mmdt, tag=f"w{l}_{k}_{m}")
                        nc.sync.dma_start(
                            t[:],
                            wl_d[li].ap()[
                                k * 128:(k + 1) * 128, m * 128:(m + 1) * 128
                            ],
                        )
                        wl_t[l, k, m] = t
            w4_t = {}
            for k in range(KTH):
                t = wp.tile([128, OUT], mmdt, tag=f"w4_{k}")
                nc.sync.dma_start(t[:], w4_d.ap()[k * 128:(k + 1) * 128, :])
                w4_t[k] = t
            bl_t = {}
            for l in range(4):
                t = wp.tile([128, MT], f32, tag=f"b{l}")
                nc.sync.dma_start(t[:], bl_d[l].ap()[:])
                bl_t[l] = t
            # bias broadcast to both m-tiles' column ranges for fused relu:
            # fused op covers (128, MT*N); bias AP must be per-partition, so
            # we keep per-m bias and slice the fused tile per m only for the
            # bias application -- i.e. still per-m ops. Instead we fuse by
            # applying relu over the 3D psum tile per m with one op each but
            # batching both m psum banks in one tile for scheduling locality.
            b4_t = wp.tile([OUT, 1], f32, tag="b4")
            nc.sync.dma_start(b4_t[:], b4_d.ap()[:])

            # ---- main loop over repeats x 512-row blocks
            for rep in range(repeat):
                for blk in range(NBLK):
                    xt = xp.tile([128, KT0, N], mmdt, tag="x")
                    nc.sync.dma_start(xt[:], xt_v[:, :, blk * N:(blk + 1) * N])

                    h_prev = None
                    for l in range(4):
                        w_tiles = w0_t if l == 0 else {
                            (k, m): wl_t[l, k, m]
                            for k in range(KTH) for m in range(MT)
                        }
                        nk = KT0 if l == 0 else KTH
                        h_cur = hp.tile([128, MT, N], mmdt, tag=f"h{l}")
                        ps = pp.tile([128, MT, N], f32, tag="ps")
                        for m in range(MT):
                            for k in range(nk):
                                rhs = xt[:, k, :] if l == 0 else h_prev[:, k, :]
                                nc.tensor.matmul(
                                    ps[:, m, :],
                                    w_tiles[k, m][:],
                                    rhs,
                                    start=(k == 0),
                                    stop=(k == nk - 1),
                                )
                        # relu(x + bias), one op per m-tile; alternate engines
                        for m in range(MT):
                            if l % 2 == 0:
                                nc.scalar.activation(
                                    h_cur[:, m, :], ps[:, m, :], RELU,
                                    bias=bl_t[l][:, m:m + 1],
                                )
                            else:
                                nc.vector.tensor_scalar(
                                    h_cur[:, m, :], ps[:, m, :],
                                    bl_t[l][:, m:m + 1], 0.0, ADD, MAX,
                                )
                        h_prev = h_cur

                    ps4 = pp4.tile([OUT, N], f32, tag="ps4")
                    for k in range(KTH):
                        nc.tensor.matmul(
                            ps4[:],
                            w4_t[k][:],
                            h_prev[:, k, :],
                            start=(k == 0),
                            stop=(k == KTH - 1),
                        )
                    ob = op.tile([OUT, N], f32, tag="ob")
                    nc.scalar.activation(ob[:], ps4[:], IDENT, bias=b4_t[:])
                    nc.sync.dma_start(out_d.ap()[:, blk * N:(blk + 1) * N], ob[:])

    nc.compile()
    return nc


def _fold_weights(W_text, b_text, W_gnn, b_gnn, W_out, b_out, adjacency, template):
    s_rows = adjacency.astype(np.float64).sum(axis=1)
    if np.ptp(s_rows) > 1e-5:
        raise ValueError("adjacency row sums are not uniform; collapse invalid")
    s = float(s_rows.mean())

    W0c = (W_text.astype(np.float64) @ (s * W_gnn[0].astype(np.float64)))
    b0c = s * (b_text.astype(np.float64) @ W_gnn[0].astype(np.float64)) + b_gnn[0]
    Wl = [s * W_gnn[l].astype(np.float64) for l in (1, 2, 3)]
    bl = [b_gnn[l] for l in (1, 2, 3)]
    W4c = np.tile(W_out, (1, 12))
    b4c = np.tile(b_out, 12) + template.reshape(36)

    mdt = _np_mm_dtype()

    def cvt(a, dt):
        return np.ascontiguousarray(np.asarray(a, dtype=np.float32).astype(dt))

    biases = [
        cvt(np.asarray(b, dtype=np.float64).reshape(MT, 128).T, np.float32)
        for b in [b0c, *bl]
    ]
    return (
        cvt(W0c, mdt), [cvt(w, mdt) for w in Wl], cvt(W4c, mdt),
        biases, cvt(np.asarray(b4c).reshape(OUT, 1), np.float32),
    )


def _make_in_maps(inputs):
    x = np.asarray(inputs["text_emb"], dtype=np.float32)
    W0c, Wl, W4c, biases, b4c = _fold_weights(
        np.asarray(inputs["W_text"]), np.asarray(inputs["b_text"]),
        np.asarray(inputs["W_gnn"]), np.asarray(inputs["b_gnn"]),
        np.asarray(inputs["W_out"]), np.asarray(inputs["b_out"]),
        np.asarray(inputs["adjacency"]), np.asarray(inputs["template"]),
    )
    mdt = _np_mm_dtype()
    in_maps = []
    for c in range(CORES):
        shard = np.ascontiguousarray(x[c * ROWS:(c + 1) * ROWS].T).astype(mdt)
        m = {"xt": shard, "w0": W0c, "w4": W4c, "b4": b4c}
        for i, l in enumerate((1, 2, 3)):
            m[f"w{l}"] = Wl[i]
        for l in range(4):
            m[f"b{l}"] = biases[l]
        in_maps.append(m)
    return in_maps


def kernel(**inputs):
    from concourse.bass_utils import run_bass_kernel_spmd

    if "nc" not in _BUILT:
        _BUILT["nc"] = _build_bass(repeat=1)
    nc = _BUILT["nc"]

    in_maps = _make_in_maps(inputs)
    res = run_bass_kernel_spmd(nc, in_maps, core_ids=list(range(CORES)))
    _BUILT["last_results"] = res
    _BUILT["last_in_maps"] = in_maps

    full = np.empty((B, OUT), dtype=np.float32)
    for c in range(CORES):
        full[c * ROWS:(c + 1) * ROWS] = res.results[c]["out"].T
    return full.reshape(B, 12, 3)


# revision 8
# speedup vs baseline: 5.8265x; 1.1678x over previous
"""MeshGNN Trainium2 kernel.

Mathematical reduction: the reference broadcasts the text projection to all 12
mesh vertices, and the row-normalized kNN adjacency has identical row sums
(every vertex has exactly K_NN=6 neighbors), so node features remain identical
across vertices through every GNN layer.  The whole network therefore
collapses to a per-row MLP:

    h   = relu(x @ W0c + b0c)          W0c = W_text @ (s*W_gnn[0])  (384,256)
    h   = relu(h @ (s*W_gnn[l]) + b_gnn[l])   l = 1..3
    o36 = h @ W4c + b4c                W4c = tile(W_out, 12) (256,36)
    out = o36.reshape(B, 12, 3)        b4c = tile(b_out,12) + template.flat

where s = 6/(6+1e-6) is the common adjacency row sum.

Device strategy (8 cores, pure data parallel over the batch):
  - host pre-transposes each core's x shard to (384, 4096) so features sit on
    SBUF partitions; all matmuls then run in feature-on-partition layout with
    weights as the stationary operand and activations as the moving operand.
  - float32r (default) or bf16 matmuls, 1 PE cycle/row at N=512.
  - relu+bias fused into one op per layer (both m-tiles at once), alternating
    ScalarE / VectorE between layers to balance the two engines.
  - output computed as (36, 4096) on device; host transposes back.
"""

import os

import numpy as np

# ---------------------------------------------------------------- constants
B = 32768
CORES = 8
ROWS = B // CORES            # 4096 rows per core
TD = 384                     # text dim
H = 256                      # hidden
OUT = 36                     # 12 verts * 3 coords
NBLK = 8                     # row blocks per core
N = ROWS // NBLK             # 512 rows per block
KT0 = TD // 128              # 3 k-tiles for layer 0
KTH = H // 128               # 2 k-tiles for hidden layers
MT = H // 128                # 2 m-tiles for hidden outputs

MM_DTYPE = os.environ.get("MESHGNN_DTYPE", "f32r")   # "f32r" | "bf16"

_BUILT = {}                  # cache: compiled Bass modules keyed by config


def _np_mm_dtype():
    if MM_DTYPE == "bf16":
        import ml_dtypes
        return ml_dtypes.bfloat16
    return np.float32


def _build_bass(repeat=1, fake_relu=False):
    """Build + compile the per-core Bass program (same NEFF on all cores).

    repeat > 1 re-runs the whole pipeline that many times inside one NEFF
    (identical outputs each pass) -- used for dispatch-free HW timing.
    fake_relu=True makes relu read a constant SBUF tile instead of PSUM
    (wrong results; PE never waits on ACT/DVE) -- PE-floor timing only.
    """
    import concourse.mybir as mybir
    import concourse.tile as tile
    from concourse import bacc

    f32 = mybir.dt.float32
    mmdt = mybir.dt.float32r if MM_DTYPE == "f32r" else mybir.dt.bfloat16
    RELU = mybir.ActivationFunctionType.Relu
    IDENT = mybir.ActivationFunctionType.Identity
    ADD = mybir.AluOpType.add
    MAX = mybir.AluOpType.max

    nc = bacc.Bacc(
        "TRN2",
        target_bir_lowering=False,
        debug=False,
        enable_asserts=False,
        num_devices=CORES,
    )

    xt_d = nc.dram_tensor("xt", (TD, ROWS), mmdt, kind="ExternalInput")
    w0_d = nc.dram_tensor("w0", (TD, H), mmdt, kind="ExternalInput")
    wl_d = [
        nc.dram_tensor(f"w{l}", (H, H), mmdt, kind="ExternalInput")
        for l in (1, 2, 3)
    ]
    w4_d = nc.dram_tensor("w4", (H, OUT), mmdt, kind="ExternalInput")
    bl_d = [
        nc.dram_tensor(f"b{l}", (128, MT), f32, kind="ExternalInput")
        for l in (0, 1, 2, 3)
    ]
    b4_d = nc.dram_tensor("b4", (OUT, 1), f32, kind="ExternalInput")
    out_d = nc.dram_tensor("out", (OUT, ROWS), f32, kind="ExternalOutput")

    # x viewed as (partition, ktile, row): row-major (TD, ROWS) split over 128
    xt_v = xt_d.ap().rearrange("(k p) n -> p k n", p=128)

    with tile.TileContext(nc) as tc:
        with (
            tc.tile_pool(name="wp", bufs=1) as wp,
            tc.tile_pool(name="xp", bufs=3) as xp,
            tc.tile_pool(name="hp", bufs=3) as hp,
            tc.tile_pool(name="op", bufs=3) as op,
            tc.tile_pool(name="pp", bufs=6, space="PSUM") as pp,
            tc.tile_pool(name="pp4", bufs=2, space="PSUM") as pp4,
        ):
            # ---- weights / biases, loaded once
            w0_t = {}
            for k in range(KT0):
                for m in range(MT):
                    t = wp.tile([128, 128], mmdt, tag=f"w0_{k}_{m}")
                    nc.sync.dma_start(
                        t[:], w0_d.ap()[k * 128:(k + 1) * 128, m * 128:(m + 1) * 128]
                    )
                    w0_t[k, m] = t
            wl_t = {}
            for li, l in enumerate((1, 2, 3)):
                for k in range(KTH):
                    for m in range(MT):
                        t = wp.tile([128, 128], mmdt, tag=f"w{l}_{k}_{m}")
                        nc.sync.dma_start(
                            t[:],
                            wl_d[li].ap()[
                                k * 128:(k + 1) * 128, m * 128:(m + 1) * 128
                            ],
                        )
                        wl_t[l, k, m] = t
            w4_t = {}
            for k in range(KTH):
                t = wp.tile([128, OUT], mmdt, tag=f"w4_{k}")
                nc.sync.dma_start(t[:], w4_d.ap()[k * 128:(k + 1) * 128, :])
                w4_t[k] = t
            bl_t = {}
            for l in range(4):
                t = wp.tile([128, MT], f32, tag=f"b{l}")
                nc.sync.dma_start(t[:], bl_d[l].ap()[:])
                bl_t[l] = t
            # bias broadcast to both m-tiles' column ranges for fused relu:
            # fused op covers (128, MT*N); bias AP must be per-partition, so
            # we keep per-m bias and slice the fused tile per m only for the
            # bias application -- i.e. still per-m ops. Instead we fuse by
            # applying relu over the 3D psum tile per m with one op each but
            # batching both m psum banks in one tile for scheduling locality.
            b4_t = wp.tile([OUT, 1], f32, tag="b4")
            nc.sync.dma_start(b4_t[:], b4_d.ap()[:])

            # ---- main loop over repeats x 512-row blocks
            for rep in range(repeat):
                for blk in range(NBLK):
                    xt = xp.tile([128, KT0, N], mmdt, tag="x")
                    nc.sync.dma_start(xt[:], xt_v[:, :, blk * N:(blk + 1) * N])

                    h_prev = None
                    for l in range(4):
                        w_tiles = w0_t if l == 0 else {
                            (k, m): wl_t[l, k, m]
                            for k in range(KTH) for m in range(MT)
                        }
                        nk = KT0 if l == 0 else KTH
                        h_cur = hp.tile([128, MT, N], mmdt, tag=f"h{l}")
                        for m in range(MT):
                            ps = pp.tile([128, N], f32, tag="ps")
                            for k in range(nk):
                                rhs = xt[:, k, :] if l == 0 else h_prev[:, k, :]
                                nc.tensor.matmul(
                                    ps[:],
                                    w_tiles[k, m][:],
                                    rhs,
                                    start=(k == 0),
                                    stop=(k == nk - 1),
                                )
                            # relu(x + bias); m0 on ScalarE, m1 on VectorE so
                            # both halves of a layer drain concurrently
                            relu_src = xt[:, 0, :] if fake_relu else ps[:]
                            if m % 2 == 0:
                                nc.scalar.activation(
                                    h_cur[:, m, :], relu_src, RELU,
                                    bias=bl_t[l][:, m:m + 1],
                                )
                            else:
                                nc.vector.tensor_scalar(
                                    h_cur[:, m, :], relu_src,
                                    bl_t[l][:, m:m + 1], 0.0, ADD, MAX,
                                )
                        h_prev = h_cur

                    ps4 = pp4.tile([OUT, N], f32, tag="ps4")
                    for k in range(KTH):
                        nc.tensor.matmul(
                            ps4[:],
                            w4_t[k][:],
                            h_prev[:, k, :],
                            start=(k == 0),
                            stop=(k == KTH - 1),
                        )
                    ob = op.tile([OUT, N], f32, tag="ob")
                    nc.scalar.activation(ob[:], ps4[:], IDENT, bias=b4_t[:])
                    nc.sync.dma_start(out_d.ap()[:, blk * N:(blk + 1) * N], ob[:])

    nc.compile()
    return nc


def _fold_weights(W_text, b_text, W_gnn, b_gnn, W_out, b_out, adjacency, template):
    s_rows = adjacency.astype(np.float64).sum(axis=1)
    if np.ptp(s_rows) > 1e-5:
        raise ValueError("adjacency row sums are not uniform; collapse invalid")
    s = float(s_rows.mean())

    W0c = (W_text.astype(np.float64) @ (s * W_gnn[0].astype(np.float64)))
    b0c = s * (b_text.astype(np.float64) @ W_gnn[0].astype(np.float64)) + b_gnn[0]
    Wl = [s * W_gnn[l].astype(np.float64) for l in (1, 2, 3)]
    bl = [b_gnn[l] for l in (1, 2, 3)]
    W4c = np.tile(W_out, (1, 12))
    b4c = np.tile(b_out, 12) + template.reshape(36)

    mdt = _np_mm_dtype()

    def cvt(a, dt):
        return np.ascontiguousarray(np.asarray(a, dtype=np.float32).astype(dt))

    biases = [
        cvt(np.asarray(b, dtype=np.float64).reshape(MT, 128).T, np.float32)
        for b in [b0c, *bl]
    ]
    return (
        cvt(W0c, mdt), [cvt(w, mdt) for w in Wl], cvt(W4c, mdt),
        biases, cvt(np.asarray(b4c).reshape(OUT, 1), np.float32),
    )


def _make_in_maps(inputs):
    x = np.asarray(inputs["text_emb"], dtype=np.float32)
    W0c, Wl, W4c, biases, b4c = _fold_weights(
        np.asarray(inputs["W_text"]), np.asarray(inputs["b_text"]),
        np.asarray(inputs["W_gnn"]), np.asarray(inputs["b_gnn"]),
        np.asarray(inputs["W_out"]), np.asarray(inputs["b_out"]),
        np.asarray(inputs["adjacency"]), np.asarray(inputs["template"]),
    )
    mdt = _np_mm_dtype()
    in_maps = []
    for c in range(CORES):
        shard = np.ascontiguousarray(x[c * ROWS:(c + 1) * ROWS].T).astype(mdt)
        m = {"xt": shard, "w0": W0c, "w4": W4c, "b4": b4c}
        for i, l in enumerate((1, 2, 3)):
            m[f"w{l}"] = Wl[i]
        for l in range(4):
            m[f"b{l}"] = biases[l]
        in_maps.append(m)
    return in_maps


def kernel(**inputs):
    from concourse.bass_utils import run_bass_kernel_spmd

    if "nc" not in _BUILT:
        _BUILT["nc"] = _build_bass(repeat=1)
    nc = _BUILT["nc"]

    in_maps = _make_in_maps(inputs)
    res = run_bass_kernel_spmd(nc, in_maps, core_ids=list(range(CORES)))
    _BUILT["last_results"] = res
    _BUILT["last_in_maps"] = in_maps

    full = np.empty((B, OUT), dtype=np.float32)
    for c in range(CORES):
        full[c * ROWS:(c + 1) * ROWS] = res.results[c]["out"].T
    return full.reshape(B, 12, 3)


# revision 9
# speedup vs baseline: 11.8906x; 2.0408x over previous
"""MeshGNN Trainium2 kernel.

Mathematical reduction: the reference broadcasts the text projection to all 12
mesh vertices, and the row-normalized kNN adjacency has identical row sums
(every vertex has exactly K_NN=6 neighbors), so node features remain identical
across vertices through every GNN layer.  The whole network therefore
collapses to a per-row MLP:

    h   = relu(x @ W0c + b0c)          W0c = W_text @ (s*W_gnn[0])  (384,256)
    h   = relu(h @ (s*W_gnn[l]) + b_gnn[l])   l = 1..3
    o36 = h @ W4c + b4c                W4c = tile(W_out, 12) (256,36)
    out = o36.reshape(B, 12, 3)        b4c = tile(b_out,12) + template.flat

where s = 6/(6+1e-6) is the common adjacency row sum.

Device strategy (8 cores, pure data parallel over the batch):
  - host pre-transposes each core's x shard to (384, 4096) so features sit on
    SBUF partitions; all matmuls then run in feature-on-partition layout with
    weights as the stationary operand and activations as the moving operand.
  - float32r (default) or bf16 matmuls, 1 PE cycle/row at N=512.
  - relu+bias fused into one op per layer (both m-tiles at once), alternating
    ScalarE / VectorE between layers to balance the two engines.
  - output computed as (36, 4096) on device; host transposes back.
"""

import os

import numpy as np

# ---------------------------------------------------------------- constants
B = 32768
CORES = 8
ROWS = B // CORES            # 4096 rows per core
TD = 384                     # text dim
H = 256                      # hidden
OUT = 36                     # 12 verts * 3 coords
NBLK = 8                     # row blocks per core
N = ROWS // NBLK             # 512 rows per block
KT0 = TD // 128              # 3 k-tiles for layer 0
KTH = H // 128               # 2 k-tiles for hidden layers
MT = H // 128                # 2 m-tiles for hidden outputs

MM_DTYPE = os.environ.get("MESHGNN_DTYPE", "f32r")   # "f32r" | "bf16"

_BUILT = {}                  # cache: compiled Bass modules keyed by config


def _np_mm_dtype():
    if MM_DTYPE == "bf16":
        import ml_dtypes
        return ml_dtypes.bfloat16
    return np.float32


def _build_bass(repeat=1, fake_relu=False):
    """Build + compile the per-core Bass program (same NEFF on all cores).

    repeat > 1 re-runs the whole pipeline that many times inside one NEFF
    (identical outputs each pass) -- used for dispatch-free HW timing.
    fake_relu=True makes relu read a constant SBUF tile instead of PSUM
    (wrong results; PE never waits on ACT/DVE) -- PE-floor timing only.
    """
    import concourse.mybir as mybir
    import concourse.tile as tile
    from concourse import bacc

    f32 = mybir.dt.float32
    mmdt = mybir.dt.float32r if MM_DTYPE == "f32r" else mybir.dt.bfloat16
    RELU = mybir.ActivationFunctionType.Relu
    IDENT = mybir.ActivationFunctionType.Identity
    ADD = mybir.AluOpType.add
    MAX = mybir.AluOpType.max

    nc = bacc.Bacc(
        "TRN2",
        target_bir_lowering=False,
        debug=False,
        enable_asserts=False,
        num_devices=CORES,
    )

    xt_d = nc.dram_tensor("xt", (TD, ROWS), mmdt, kind="ExternalInput")
    w0_d = nc.dram_tensor("w0", (TD, H), mmdt, kind="ExternalInput")
    wl_d = [
        nc.dram_tensor(f"w{l}", (H, H), mmdt, kind="ExternalInput")
        for l in (1, 2, 3)
    ]
    w4_d = nc.dram_tensor("w4", (H, OUT), mmdt, kind="ExternalInput")
    bl_d = [
        nc.dram_tensor(f"b{l}", (128, MT), f32, kind="ExternalInput")
        for l in (0, 1, 2, 3)
    ]
    b4_d = nc.dram_tensor("b4", (OUT, 1), f32, kind="ExternalInput")
    out_d = nc.dram_tensor("out", (OUT, ROWS), f32, kind="ExternalOutput")

    # x viewed as (partition, ktile, row): row-major (TD, ROWS) split over 128
    xt_v = xt_d.ap().rearrange("(k p) n -> p k n", p=128)

    with tile.TileContext(nc) as tc:
        with (
            tc.tile_pool(name="wp", bufs=1) as wp,
            tc.tile_pool(name="xp", bufs=3) as xp,
            tc.tile_pool(name="hp", bufs=3) as hp,
            tc.tile_pool(name="op", bufs=3) as op,
            tc.tile_pool(name="pp", bufs=6, space="PSUM") as pp,
            tc.tile_pool(name="pp4", bufs=2, space="PSUM") as pp4,
        ):
            # ---- weights / biases, loaded once
            w0_t = {}
            for k in range(KT0):
                for m in range(MT):
                    t = wp.tile([128, 128], mmdt, tag=f"w0_{k}_{m}")
                    nc.sync.dma_start(
                        t[:], w0_d.ap()[k * 128:(k + 1) * 128, m * 128:(m + 1) * 128]
                    )
                    w0_t[k, m] = t
            wl_t = {}
            for li, l in enumerate((1, 2, 3)):
                for k in range(KTH):
                    for m in range(MT):
                        t = wp.tile([128, 128], mmdt, tag=f"w{l}_{k}_{m}")
                        nc.sync.dma_start(
                            t[:],
                            wl_d[li].ap()[
                                k * 128:(k + 1) * 128, m * 128:(m + 1) * 128
                            ],
                        )
                        wl_t[l, k, m] = t
            w4_t = {}
            for k in range(KTH):
                t = wp.tile([128, OUT], mmdt, tag=f"w4_{k}")
                nc.sync.dma_start(t[:], w4_d.ap()[k * 128:(k + 1) * 128, :])
                w4_t[k] = t
            bl_t = {}
            for l in range(4):
                t = wp.tile([128, MT], f32, tag=f"b{l}")
                nc.sync.dma_start(t[:], bl_d[l].ap()[:])
                bl_t[l] = t
            # bias broadcast to both m-tiles' column ranges for fused relu:
            # fused op covers (128, MT*N); bias AP must be per-partition, so
            # we keep per-m bias and slice the fused tile per m only for the
            # bias application -- i.e. still per-m ops. Instead we fuse by
            # applying relu over the 3D psum tile per m with one op each but
            # batching both m psum banks in one tile for scheduling locality.
            b4_t = wp.tile([OUT, 1], f32, tag="b4")
            nc.sync.dma_start(b4_t[:], b4_d.ap()[:])

            # ---- main loop over repeats x pairs of 512-row blocks.
            # Two blocks are interleaved layer-by-layer so the PE always has
            # an independent matmul stream while the other block's relu
            # drains; per-k x DMAs let L0 start on the first k-tile.
            for rep in range(repeat):
                for pair in range(NBLK // 2):
                    blks = (2 * pair, 2 * pair + 1)
                    xts = {}
                    for b in blks:
                        xt = xp.tile([128, KT0, N], mmdt, tag=f"x{b % 2}")
                        for k in range(KT0):
                            nc.sync.dma_start(
                                xt[:, k, :],
                                xt_v[:, k, b * N:(b + 1) * N],
                            )
                        xts[b] = xt

                    h_prev = {b: None for b in blks}
                    for l in range(4):
                        w_tiles = w0_t if l == 0 else {
                            (k, m): wl_t[l, k, m]
                            for k in range(KTH) for m in range(MT)
                        }
                        nk = KT0 if l == 0 else KTH
                        for b in blks:
                            h_cur = hp.tile(
                                [128, MT, N], mmdt, tag=f"h{l}{b % 2}"
                            )
                            for m in range(MT):
                                ps = pp.tile([128, N], f32, tag="ps")
                                for k in range(nk):
                                    rhs = (
                                        xts[b][:, k, :] if l == 0
                                        else h_prev[b][:, k, :]
                                    )
                                    nc.tensor.matmul(
                                        ps[:],
                                        w_tiles[k, m][:],
                                        rhs,
                                        start=(k == 0),
                                        stop=(k == nk - 1),
                                    )
                                # relu(x+bias); m0 on ScalarE, m1 on VectorE
                                relu_src = ps[:]
                                if m % 2 == 0:
                                    nc.scalar.activation(
                                        h_cur[:, m, :], relu_src, RELU,
                                        bias=bl_t[l][:, m:m + 1],
                                    )
                                else:
                                    nc.vector.tensor_scalar(
                                        h_cur[:, m, :], relu_src,
                                        bl_t[l][:, m:m + 1], 0.0, ADD, MAX,
                                    )
                            h_prev[b] = h_cur

                    for b in blks:
                        ps4 = pp4.tile([OUT, N], f32, tag="ps4")
                        for k in range(KTH):
                            nc.tensor.matmul(
                                ps4[:],
                                w4_t[k][:],
                                h_prev[b][:, k, :],
                                start=(k == 0),
                                stop=(k == KTH - 1),
                            )
                        ob = op.tile([OUT, N], f32, tag="ob")
                        nc.scalar.activation(ob[:], ps4[:], IDENT, bias=b4_t[:])
                        nc.sync.dma_start(
                            out_d.ap()[:, b * N:(b + 1) * N], ob[:]
                        )

    nc.compile()
    return nc


def _fold_weights(W_text, b_text, W_gnn, b_gnn, W_out, b_out, adjacency, template):
    s_rows = adjacency.astype(np.float64).sum(axis=1)
    if np.ptp(s_rows) > 1e-5:
        raise ValueError("adjacency row sums are not uniform; collapse invalid")
    s = float(s_rows.mean())

    W0c = (W_text.astype(np.float64) @ (s * W_gnn[0].astype(np.float64)))
    b0c = s * (b_text.astype(np.float64) @ W_gnn[0].astype(np.float64)) + b_gnn[0]
    Wl = [s * W_gnn[l].astype(np.float64) for l in (1, 2, 3)]
    bl = [b_gnn[l] for l in (1, 2, 3)]
    W4c = np.tile(W_out, (1, 12))
    b4c = np.tile(b_out, 12) + template.reshape(36)

    mdt = _np_mm_dtype()

    def cvt(a, dt):
        return np.ascontiguousarray(np.asarray(a, dtype=np.float32).astype(dt))

    biases = [
        cvt(np.asarray(b, dtype=np.float64).reshape(MT, 128).T, np.float32)
        for b in [b0c, *bl]
    ]
    return (
        cvt(W0c, mdt), [cvt(w, mdt) for w in Wl], cvt(W4c, mdt),
        biases, cvt(np.asarray(b4c).reshape(OUT, 1), np.float32),
    )


def _make_in_maps(inputs):
    x = np.asarray(inputs["text_emb"], dtype=np.float32)
    W0c, Wl, W4c, biases, b4c = _fold_weights(
        np.asarray(inputs["W_text"]), np.asarray(inputs["b_text"]),
        np.asarray(inputs["W_gnn"]), np.asarray(inputs["b_gnn"]),
        np.asarray(inputs["W_out"]), np.asarray(inputs["b_out"]),
        np.asarray(inputs["adjacency"]), np.asarray(inputs["template"]),
    )
    mdt = _np_mm_dtype()
    in_maps = []
    for c in range(CORES):
        shard = np.ascontiguousarray(x[c * ROWS:(c + 1) * ROWS].T).astype(mdt)
        m = {"xt": shard, "w0": W0c, "w4": W4c, "b4": b4c}
        for i, l in enumerate((1, 2, 3)):
            m[f"w{l}"] = Wl[i]
        for l in range(4):
            m[f"b{l}"] = biases[l]
        in_maps.append(m)
    return in_maps


def kernel(**inputs):
    from concourse.bass_utils import run_bass_kernel_spmd

    if "nc" not in _BUILT:
        _BUILT["nc"] = _build_bass(repeat=1)
    nc = _BUILT["nc"]

    in_maps = _make_in_maps(inputs)
    res = run_bass_kernel_spmd(nc, in_maps, core_ids=list(range(CORES)))
    _BUILT["last_results"] = res
    _BUILT["last_in_maps"] = in_maps

    full = np.empty((B, OUT), dtype=np.float32)
    for c in range(CORES):
        full[c * ROWS:(c + 1) * ROWS] = res.results[c]["out"].T
    return full.reshape(B, 12, 3)
